# revision 17
# baseline (speedup 1.0000x reference)
"""Guided channel-wise 3x3 conv (per-pixel weights) on 8 Trainium2 cores.

out[b,c,h,w] = sum_{dh,dw in {-1,0,1}} input[b,c,h+dh,w+dw] * weights[b,c,k(dh,dw),h,w]
with SAME zero padding.  Shapes: input (8,64,128,128) f32,
weights (8,64,9,128,128) f32 -> out (8,64,128,128) f32.

Sharding: pure data parallelism, one batch sample per NeuronCore (B=8 cores).

Per-core layout: 128 SBUF partitions = (half, c) with p = half*64 + c; each
partition holds one 64-row half of one channel plane (input padded 66x130).
Everything on-chip is fp16: halves HBM traffic (memory-bound regime) and
doubles DVE throughput (2x_1p mode needs 2-byte packed operands); rel err
~6e-4 vs the 2e-2 gate.

The DVE does 9 multiplies + 8 accumulates (~75 us of engine time); the
pipeline hides DMA behind it:
  - taps 0-2 are processed in 16-row quarters gated on quarter-granular
    weight DMAs, so compute starts as soon as the first ~1 MB lands and the
    serial fill (input + first taps) is overlapped,
  - taps 3-8 stream whole planes through 4 buffer slots (DMA runs ahead),
  - the final accumulate runs in quarters, each immediately flushed to HBM.

DMA completions are OUT OF ORDER on this hardware (queue packets fan out
over 16 DMA engines), so a single cumulative DMA semaphore is unsound.
Every awaited transfer group gets a private semaphore, and consumers wait
for that semaphore's full count — correct under any completion order.
"""

import numpy as np

from concourse import bass, mybir
from concourse.bass_utils import run_bass_kernel_spmd

B, CI, H, W = 8, 64, 128, 128
K = 9
HH = H // 2  # rows per half-plane (64)
PR = HH + 2  # padded rows per partition (66)
PC = W + 2  # padded cols (130)
NP = 128  # SBUF partitions
FP = HH * W  # free elems per partition of one output half-plane (8192)
QF = FP // 4  # quarter free elems (2048)
QR = HH // 4  # quarter rows (16)

F16 = mybir.dt.float16

TAPS = [4, 0, 1, 2, 3, 5, 6, 7, 8]  # center tap first: it initializes out
NSLOT = 4
NQTAP = 3  # taps processed in quarters (0..NQTAP-1)

# input DMA pieces: padded row ranges (disjoint); piece q covers the rows
# needed by quarter q of any tap (16q+dh .. 16q+16+dh, dh<=2)
IN_PIECES = [(0, 18), (18, 34), (34, 50), (50, 66)]


def build_bass():
    nc = bass.Bass()
    inp = nc.declare_dram_parameter("input", [NP, PR * PC], F16, isOutput=False)
    wts = nc.declare_dram_parameter("weights", [K, NP, FP], F16, isOutput=False)
    wq8d = nc.declare_dram_parameter("wq8", [NQTAP, NP, FP], mybir.dt.int8, isOutput=False)
    wscd = nc.declare_dram_parameter("wscale", [NP, NQTAP], mybir.dt.float32, isOutput=False)
    out = nc.declare_dram_parameter("out", [NP, FP], F16, isOutput=True)

    from contextlib import ExitStack

    with ExitStack() as ctx:
        in_pad = ctx.enter_context(nc.sbuf_tensor("in_pad", [NP, PR * PC], F16))
        wt = [
            ctx.enter_context(nc.sbuf_tensor(f"wt{i}", [NP, FP], F16))
            for i in range(NSLOT)
        ]
        wq8_sb = ctx.enter_context(nc.sbuf_tensor("wq8_sb", [NP, NQTAP * FP], mybir.dt.int8))
        wsc_sb = ctx.enter_context(nc.sbuf_tensor("wsc_sb", [NP, NQTAP], mybir.dt.float32))
        tmp = ctx.enter_context(nc.sbuf_tensor("tmp", [NP, FP], F16))
        out_t = ctx.enter_context(nc.sbuf_tensor("out_t", [NP, FP], F16))
        block = ctx.enter_context(nc.Block())
        in_sems = [
            ctx.enter_context(nc.semaphore(f"in_sem{q}")) for q in range(4)
        ]
        # private per-tap weight sems; quartered int8 taps get one per quarter
        wq_sems = {
            (j, q): ctx.enter_context(nc.semaphore(f"w{j}q{q}_sem"))
            for j in range(NQTAP)
            for q in range(4)
        }
        w_sems = {
            j: ctx.enter_context(nc.semaphore(f"w{j}_sem"))
            for j in range(NQTAP, K)
        }
        wsc_sem = ctx.enter_context(nc.semaphore("wsc_sem"))
        act_sem = ctx.enter_context(nc.semaphore("act_sem"))
        dve_sem = ctx.enter_context(nc.semaphore("dve_sem"))
        out_sem = ctx.enter_context(nc.semaphore("out_sem"))

        in3 = in_pad[:].rearrange("p (r w) -> p r w", r=PR)
        out3 = out_t[:].rearrange("p (r w) -> p r w", r=HH)
        tmp3 = tmp[:].rearrange("p (r w) -> p r w", r=HH)

        # dve_sem: +1 after the LAST weight-read (final mult) of each tap
        # (slot-reuse gate), then +1 per final-accumulate quarter (out gate).

        @block.sync
        def _(sync):
            sync.dma_start(out=wsc_sb[:], in_=wscd[:]).then_inc(wsc_sem, 16)
            # interleave input pieces with tap-0 int8 weight quarters
            for q in range(4):
                r0, r1 = IN_PIECES[q]
                sync.dma_start(
                    out=in_pad[:, r0 * PC : r1 * PC],
                    in_=inp[:, r0 * PC : r1 * PC],
                ).then_inc(in_sems[q], 16)
                sync.dma_start(
                    out=wq8_sb[:, q * QF : (q + 1) * QF],
                    in_=wq8d[0, :, q * QF : (q + 1) * QF],
                ).then_inc(wq_sems[(0, q)], 16)
            for j in range(1, NQTAP):
                for q in range(4):
                    sync.dma_start(
                        out=wq8_sb[:, j * FP + q * QF : j * FP + (q + 1) * QF],
                        in_=wq8d[j, :, q * QF : (q + 1) * QF],
                    ).then_inc(wq_sems[(j, q)], 16)
            for j in range(NQTAP, K):
                if j >= NSLOT:
                    sync.wait_ge(dve_sem, j - NSLOT + 1)
                sync.dma_start(out=wt[j % NSLOT][:], in_=wts[TAPS[j]]).then_inc(
                    w_sems[j], 16
                )
            for q in range(8):
                sync.wait_ge(dve_sem, K + q + 1)
                sync.dma_start(
                    out=out[:, q * (QF // 2) : (q + 1) * (QF // 2)],
                    in_=out_t[:, q * (QF // 2) : (q + 1) * (QF // 2)],
                ).then_inc(out_sem, 16)
            sync.wait_ge(out_sem, 128)

        @block.scalar
        def _(scalar):
            # dequantize the early int8 taps into their fp16 slots, folding
            # the per-(partition, tap) scale back in via the activation scale
            scalar.wait_ge(wsc_sem, 16)
            for j in range(NQTAP):
                for q in range(4):
                    scalar.wait_ge(wq_sems[(j, q)], 16)
                    scalar.activation(
                        out=wt[j][:, q * QF : (q + 1) * QF],
                        in_=wq8_sb[:, j * FP + q * QF : j * FP + (q + 1) * QF],
                        func=mybir.ActivationFunctionType.Copy,
                        scale=wsc_sb[:, j : j + 1],
                    ).then_inc(act_sem, 1)

        @block.vector
        def _(vector):
            for j in range(K):
                k = TAPS[j]
                dh, dw = k // 3, k % 3
                wt3 = wt[j % NSLOT][:].rearrange("p (r w) -> p r w", r=HH)
                if j < NQTAP:
                    # quarter-granular: mult (and for j>0 accumulate) per 16 rows
                    for q in range(4):
                        if j == 0:
                            vector.wait_ge(in_sems[q], 16)
                        vector.wait_ge(act_sem, 4 * j + q + 1)
                        r = q * QR
                        i0 = in3[:, r + dh : r + dh + QR, dw : dw + W]
                        if j == 0:
                            mm = vector.tensor_tensor(
                                out=out3[:, r : r + QR],
                                in0=i0,
                                in1=wt3[:, r : r + QR],
                                op=mybir.AluOpType.mult,
                            )
                        else:
                            mm = vector.tensor_tensor(
                                out=tmp3[:, r : r + QR],
                                in0=i0,
                                in1=wt3[:, r : r + QR],
                                op=mybir.AluOpType.mult,
                            )
                        if q == 3:
                            mm.then_inc(dve_sem, 1)
                        if j > 0:
                            vector.tensor_tensor(
                                out=out3[:, r : r + QR],
                                in0=out3[:, r : r + QR],
                                in1=tmp3[:, r : r + QR],
                                op=mybir.AluOpType.add,
                            )
                    continue
                vector.wait_ge(w_sems[j], 16)
                vector.tensor_tensor(
                    out=tmp3,
                    in0=in3[:, dh : dh + HH, dw : dw + W],
                    in1=wt3,
                    op=mybir.AluOpType.mult,
                ).then_inc(dve_sem, 1)
                if j == K - 1:
                    # final accumulate in eighths; each releases an out DMA
                    for q in range(8):
                        r = q * (QR // 2)
                        vector.tensor_tensor(
                            out=out3[:, r : r + QR // 2],
                            in0=out3[:, r : r + QR // 2],
                            in1=tmp3[:, r : r + QR // 2],
                            op=mybir.AluOpType.add,
                        ).then_inc(dve_sem, 1)
                else:
                    vector.tensor_tensor(
                        out=out3, in0=out3, in1=tmp3, op=mybir.AluOpType.add
                    )

    return nc


def _prep_input(x):
    """(64,128,128) f32 -> (128, 66*130) fp16 per-partition padded layout."""
    pad = np.zeros((CI, H + 2, W + 2), dtype=np.float16)
    pad[:, 1 : H + 1, 1 : W + 1] = x.astype(np.float16)
    win = np.stack([pad[:, 0:PR, :], pad[:, HH : HH + PR, :]], axis=0)
    return np.ascontiguousarray(win.reshape(NP, PR * PC))


def _prep_weights(w):
    """(64,9,128,128) f32 -> fp16 (9,128,8192), int8 early taps, scales.

    Partition p = half*64 + c.  Taps TAPS[0..NQTAP-1] are also emitted as
    per-(partition, tap) scaled int8 (those fp16 planes are never read).
    """
    wr = w.reshape(CI, K, 2, HH, W).transpose(1, 2, 0, 3, 4).reshape(K, NP, FP)
    wf16 = np.ascontiguousarray(wr.astype(np.float16))
    wq8 = np.empty((NQTAP, NP, FP), dtype=np.int8)
    wsc = np.empty((NP, NQTAP), dtype=np.float32)
    for j in range(NQTAP):
        plane = wr[TAPS[j]].astype(np.float32)  # (128, 8192)
        s = np.maximum(np.abs(plane).max(axis=1) / 127.0, 1e-12)
        wq8[j] = np.clip(np.round(plane / s[:, None]), -127, 127).astype(np.int8)
        wsc[:, j] = s
    return wf16, np.ascontiguousarray(wq8), np.ascontiguousarray(wsc)


def _unprep_out(o):
    """(128, 64*128) fp16 -> (64,128,128) f32."""
    return np.ascontiguousarray(
        o.astype(np.float32).reshape(2, CI, HH, W).transpose(1, 0, 2, 3).reshape(CI, H, W)
    )


_NC = None


def _get_nc():
    global _NC
    if _NC is None:
        _NC = build_bass()
    return _NC


def make_in_maps(input, weights):
    input = np.asarray(input, dtype=np.float32)
    weights = np.asarray(weights, dtype=np.float32)
    maps = []
    for b in range(B):
        wf16, wq8, wsc = _prep_weights(weights[b])
        maps.append(
            {
                "input": _prep_input(input[b]),
                "weights": wf16,
                "wq8": wq8,
                "wscale": wsc,
            }
        )
    return maps


def kernel(input, weights):
    nc = _get_nc()
    in_maps = make_in_maps(input, weights)
    res = run_bass_kernel_spmd(nc, in_maps, list(range(B)))
    return np.stack([_unprep_out(res.results[b]["out"]) for b in range(B)], axis=0)


# revision 18
# speedup vs baseline: 1.0238x; 1.0238x over previous
"""Guided channel-wise 3x3 conv (per-pixel weights) on 8 Trainium2 cores.

out[b,c,h,w] = sum_{dh,dw in {-1,0,1}} input[b,c,h+dh,w+dw] * weights[b,c,k(dh,dw),h,w]
with SAME zero padding.  Shapes: input (8,64,128,128) f32,
weights (8,64,9,128,128) f32 -> out (8,64,128,128) f32.

Sharding: pure data parallelism, one batch sample per NeuronCore (B=8 cores).

Per-core layout: 128 SBUF partitions = (half, c) with p = half*64 + c; each
partition holds one 64-row half of one channel plane (input padded 66x130).
Everything on-chip is fp16: halves HBM traffic (memory-bound regime) and
doubles DVE throughput (2x_1p mode needs 2-byte packed operands); rel err
~6e-4 vs the 2e-2 gate.

The DVE does 9 multiplies + 8 accumulates (~75 us of engine time); the
pipeline hides DMA behind it:
  - taps 0-2 are processed in 16-row quarters gated on quarter-granular
    weight DMAs, so compute starts as soon as the first ~1 MB lands and the
    serial fill (input + first taps) is overlapped,
  - taps 3-8 stream whole planes through 4 buffer slots (DMA runs ahead),
  - the final accumulate runs in quarters, each immediately flushed to HBM.

DMA completions are OUT OF ORDER on this hardware (queue packets fan out
over 16 DMA engines), so a single cumulative DMA semaphore is unsound.
Every awaited transfer group gets a private semaphore, and consumers wait
for that semaphore's full count — correct under any completion order.
"""

import numpy as np

from concourse import bass, mybir
from concourse.bass_utils import run_bass_kernel_spmd

B, CI, H, W = 8, 64, 128, 128
K = 9
HH = H // 2  # rows per half-plane (64)
PR = HH + 2  # padded rows per partition (66)
PC = W + 2  # padded cols (130)
NP = 128  # SBUF partitions
FP = HH * W  # free elems per partition of one output half-plane (8192)
QF = FP // 4  # quarter free elems (2048)
QR = HH // 4  # quarter rows (16)

F16 = mybir.dt.float16

TAPS = [4, 0, 1, 2, 3, 5, 6, 7, 8]  # center tap first: it initializes out
NSLOT = 4
NQTAP = 3  # taps processed in quarters (0..NQTAP-1)

# input DMA pieces: padded row ranges (disjoint); piece q covers the rows
# needed by quarter q of any tap (16q+dh .. 16q+16+dh, dh<=2)
IN_PIECES = [(0, 18), (18, 34), (34, 50), (50, 66)]


def build_bass():
    nc = bass.Bass()
    inp = nc.declare_dram_parameter("input", [NP, PR * PC], F16, isOutput=False)
    wts = nc.declare_dram_parameter("weights", [K, NP, FP], F16, isOutput=False)
    out = nc.declare_dram_parameter("out", [NP, FP], F16, isOutput=True)

    from contextlib import ExitStack

    with ExitStack() as ctx:
        in_pad = ctx.enter_context(nc.sbuf_tensor("in_pad", [NP, PR * PC], F16))
        wt = [
            ctx.enter_context(nc.sbuf_tensor(f"wt{i}", [NP, FP], F16))
            for i in range(NSLOT)
        ]
        tmp = ctx.enter_context(nc.sbuf_tensor("tmp", [NP, FP], F16))
        out_t = ctx.enter_context(nc.sbuf_tensor("out_t", [NP, FP], F16))
        block = ctx.enter_context(nc.Block())
        in_sems = [
            ctx.enter_context(nc.semaphore(f"in_sem{q}")) for q in range(4)
        ]
        # private per-tap weight sems; quartered int8 taps get one per quarter
        wq_sems = {
            (j, q): ctx.enter_context(nc.semaphore(f"w{j}q{q}_sem"))
            for j in range(NQTAP)
            for q in range(4)
        }
        w_sems = {
            j: ctx.enter_context(nc.semaphore(f"w{j}_sem"))
            for j in range(NQTAP, K)
        }
        dve_sem = ctx.enter_context(nc.semaphore("dve_sem"))
        out_sem = ctx.enter_context(nc.semaphore("out_sem"))

        in3 = in_pad[:].rearrange("p (r w) -> p r w", r=PR)
        out3 = out_t[:].rearrange("p (r w) -> p r w", r=HH)
        tmp3 = tmp[:].rearrange("p (r w) -> p r w", r=HH)

        # dve_sem: +1 after the LAST weight-read (final mult) of each tap
        # (slot-reuse gate), then +1 per final-accumulate quarter (out gate).

        @block.sync
        def _(sync):
            # interleave input pieces with tap-0 weight quarters
            for q in range(4):
                r0, r1 = IN_PIECES[q]
                sync.dma_start(
                    out=in_pad[:, r0 * PC : r1 * PC],
                    in_=inp[:, r0 * PC : r1 * PC],
                ).then_inc(in_sems[q], 16)
                sync.dma_start(
                    out=wt[0][:, q * QF : (q + 1) * QF],
                    in_=wts[TAPS[0], :, q * QF : (q + 1) * QF],
                ).then_inc(wq_sems[(0, q)], 16)
            for j in range(1, NQTAP):
                for q in range(4):
                    sync.dma_start(
                        out=wt[j][:, q * QF : (q + 1) * QF],
                        in_=wts[TAPS[j], :, q * QF : (q + 1) * QF],
                    ).then_inc(wq_sems[(j, q)], 16)
            for j in range(NQTAP, K):
                if j >= NSLOT:
                    sync.wait_ge(dve_sem, j - NSLOT + 1)
                sync.dma_start(out=wt[j % NSLOT][:], in_=wts[TAPS[j]]).then_inc(
                    w_sems[j], 16
                )
            for q in range(8):
                sync.wait_ge(dve_sem, K + q + 1)
                sync.dma_start(
                    out=out[:, q * (QF // 2) : (q + 1) * (QF // 2)],
                    in_=out_t[:, q * (QF // 2) : (q + 1) * (QF // 2)],
                ).then_inc(out_sem, 16)
            sync.wait_ge(out_sem, 128)

        @block.vector
        def _(vector):
            for j in range(K):
                k = TAPS[j]
                dh, dw = k // 3, k % 3
                wt3 = wt[j % NSLOT][:].rearrange("p (r w) -> p r w", r=HH)
                if j < NQTAP:
                    # quarter-granular: mult (and for j>0 accumulate) per 16 rows
                    for q in range(4):
                        if j == 0:
                            vector.wait_ge(in_sems[q], 16)
                        vector.wait_ge(wq_sems[(j, q)], 16)
                        r = q * QR
                        i0 = in3[:, r + dh : r + dh + QR, dw : dw + W]
                        if j == 0:
                            mm = vector.tensor_tensor(
                                out=out3[:, r : r + QR],
                                in0=i0,
                                in1=wt3[:, r : r + QR],
                                op=mybir.AluOpType.mult,
                            )
                        else:
                            mm = vector.tensor_tensor(
                                out=tmp3[:, r : r + QR],
                                in0=i0,
                                in1=wt3[:, r : r + QR],
                                op=mybir.AluOpType.mult,
                            )
                        if q == 3:
                            mm.then_inc(dve_sem, 1)
                        if j > 0:
                            vector.tensor_tensor(
                                out=out3[:, r : r + QR],
                                in0=out3[:, r : r + QR],
                                in1=tmp3[:, r : r + QR],
                                op=mybir.AluOpType.add,
                            )
                    continue
                vector.wait_ge(w_sems[j], 16)
                vector.tensor_tensor(
                    out=tmp3,
                    in0=in3[:, dh : dh + HH, dw : dw + W],
                    in1=wt3,
                    op=mybir.AluOpType.mult,
                ).then_inc(dve_sem, 1)
                if j == K - 1:
                    # final accumulate in eighths; each releases an out DMA
                    for q in range(8):
                        r = q * (QR // 2)
                        vector.tensor_tensor(
                            out=out3[:, r : r + QR // 2],
                            in0=out3[:, r : r + QR // 2],
                            in1=tmp3[:, r : r + QR // 2],
                            op=mybir.AluOpType.add,
                        ).then_inc(dve_sem, 1)
                else:
                    vector.tensor_tensor(
                        out=out3, in0=out3, in1=tmp3, op=mybir.AluOpType.add
                    )

    return nc


def _prep_input(x):
    """(64,128,128) f32 -> (128, 66*130) fp16 per-partition padded layout."""
    pad = np.zeros((CI, H + 2, W + 2), dtype=np.float16)
    pad[:, 1 : H + 1, 1 : W + 1] = x.astype(np.float16)
    win = np.stack([pad[:, 0:PR, :], pad[:, HH : HH + PR, :]], axis=0)
    return np.ascontiguousarray(win.reshape(NP, PR * PC))


def _prep_weights(w):
    """(64,9,128,128) f32 -> (9, 128, 64*128) fp16, partition p = half*64 + c."""
    wr = w.astype(np.float16).reshape(CI, K, 2, HH, W).transpose(1, 2, 0, 3, 4)
    return np.ascontiguousarray(wr.reshape(K, NP, FP))


def _unprep_out(o):
    """(128, 64*128) fp16 -> (64,128,128) f32."""
    return np.ascontiguousarray(
        o.astype(np.float32).reshape(2, CI, HH, W).transpose(1, 0, 2, 3).reshape(CI, H, W)
    )


_NC = None


def _get_nc():
    global _NC
    if _NC is None:
        _NC = build_bass()
    return _NC


def make_in_maps(input, weights):
    input = np.asarray(input, dtype=np.float32)
    weights = np.asarray(weights, dtype=np.float32)
    return [
        {"input": _prep_input(input[b]), "weights": _prep_weights(weights[b])}
        for b in range(B)
    ]


def kernel(input, weights):
    nc = _get_nc()
    in_maps = make_in_maps(input, weights)
    res = run_bass_kernel_spmd(nc, in_maps, list(range(B)))
    return np.stack([_unprep_out(res.results[b]["out"]) for b in range(B)], axis=0)
